# revision 7
# baseline (speedup 1.0000x reference)
"""Expert-parallel MoE (top-1, E=8, C=2048, D=1024, H=4096) on 8 TRN2 cores.

Strategy (expert-parallel, per sharding hint):
  - Every core receives the FULL x and computes the routing (gate fp32,
    argmax, capacity-aware positions) redundantly. Core e owns expert e:
    W1[e]/b1[e]/W2[e]/b2[e] only.
  - Routing positions are computed with triangular-matmul cumsums; the
    per-expert gather/scatter index tables are built with indicator-matrix
    matmuls (no serial scatter).
  - Dispatch: SWDGE dma_gather of the expert's token rows (fp32r).
  - Expert MLP runs in fp32r (TF32) at full PE rate: x^T tiles produced by
    PE transpose; GEMM1 -> relu(+b1) on ACT -> GEMM2 (+b2) accumulated over
    H-blocks into an SBUF y buffer.
  - Combine: dma_scatter_add of y rows into a zero-initialized [N+1, D]
    output (row N is a trash row for empty slots); dropped tokens are never
    scattered and stay zero. Host unshard = sum of the 8 disjoint outputs.
"""

import sys

sys.path.insert(0, "/opt/trn_rl_repo")

import numpy as np

N = 8192          # tokens
D = 1024          # model dim
E = 8             # experts
H = 4096          # hidden
C = 2048          # per-expert capacity
NT = N // 128     # 64 token tiles
MC = 2            # megachunks over slots
MCT = C // MC     # 1024 slots per megachunk
HB = 512          # H-block size
NHB = H // HB     # 8
NCORE = 8

_CACHE = {}


def _build(debug=False):
    import concourse.bacc as bacc
    import concourse.bass as bass
    import concourse.tile as tile
    import concourse.mybir as mybir

    F32 = mybir.dt.float32
    F32R = mybir.dt.float32r
    I16 = mybir.dt.int16
    OP = mybir.AluOpType
    AF = mybir.ActivationFunctionType
    AX = mybir.AxisListType

    nc = bacc.Bacc("TRN2", target_bir_lowering=False, debug=False,
                   num_devices=NCORE)

    # ---- I/O ----
    d_x = nc.dram_tensor("x", [N, D], F32, kind="ExternalInput").ap()
    d_xr = nc.dram_tensor("xr", [N, D], F32R, kind="ExternalInput").ap()
    d_w1 = nc.dram_tensor("w1", [D, H], F32R, kind="ExternalInput").ap()
    d_w2 = nc.dram_tensor("w2", [H, D], F32R, kind="ExternalInput").ap()
    d_b1 = nc.dram_tensor("b1l", [128, H // 128], F32, kind="ExternalInput").ap()
    d_b2 = nc.dram_tensor("b2r", [1, D], F32R, kind="ExternalInput").ap()
    d_wg = nc.dram_tensor("wg", [128, D // 128, E], F32, kind="ExternalInput").ap()
    d_bg = nc.dram_tensor("bgrep", [128, E], F32, kind="ExternalInput").ap()
    d_idn = nc.dram_tensor("idn", [128, 128], F32, kind="ExternalInput").ap()
    d_idr = nc.dram_tensor("idr", [128, 128], F32R, kind="ExternalInput").ap()
    d_ut = nc.dram_tensor("ut128", [128, 128], F32, kind="ExternalInput").ap()
    d_u64 = nc.dram_tensor("u64", [64, 64], F32, kind="ExternalInput").ap()
    d_on128 = nc.dram_tensor("on128", [128, 1], F32, kind="ExternalInput").ap()
    d_on1r = nc.dram_tensor("on1r", [1, 128], F32R, kind="ExternalInput").ap()
    d_io8 = nc.dram_tensor("io8", [128, E], F32, kind="ExternalInput").ap()
    d_de8 = nc.dram_tensor("de8", [128, E], F32, kind="ExternalInput").ap()
    d_io16 = nc.dram_tensor("io16", [128, 32], F32, kind="ExternalInput").ap()
    d_flo = nc.dram_tensor("flo", [128, 128], F32, kind="ExternalInput").ap()
    d_fhi = nc.dram_tensor("fhi", [128, 128], F32, kind="ExternalInput").ap()
    d_fix = nc.dram_tensor("fix", [128, 128], F32, kind="ExternalInput").ap()
    d_tok = nc.dram_tensor("tokid", [128, NT], F32, kind="ExternalInput").ap()
    d_ev = nc.dram_tensor("evec", [128, 1], F32, kind="ExternalInput").ap()

    d_out = nc.dram_tensor("out", [N + 1, D], F32, kind="ExternalOutput").ap()
    if debug:
        d_dbg_eid = nc.dram_tensor("dbg_eid", [128, NT], F32, kind="ExternalOutput").ap()
        d_dbg_cnt = nc.dram_tensor("dbg_cnt", [1, NT * E], F32, kind="ExternalOutput").ap()
        d_dbg_car = nc.dram_tensor("dbg_car", [128, NT * E], F32, kind="ExternalOutput").ap()
        d_dbg_gidx = nc.dram_tensor("dbg_gidx", [128, C // 16], I16, kind="ExternalOutput").ap()
        d_dbg_sidx = nc.dram_tensor("dbg_sidx", [128, C // 16], I16, kind="ExternalOutput").ap()
        d_dbg_disp = nc.dram_tensor("dbg_disp", [128, MCT // 128, D], F32, kind="ExternalOutput").ap()

    with tile.TileContext(nc) as tc:
        with (
            tc.tile_pool(name="sb", bufs=1) as pool,
            tc.tile_pool(name="ps", bufs=1, space="PSUM") as psp,
            tc.tile_pool(name="dr", bufs=1, space="DRAM") as drp,
        ):
            # ---- consts ----
            c_idn = pool.tile([128, 128], F32, tag="c_idn")
            c_idr = pool.tile([128, 128], F32R, tag="c_idr")
            c_ut = pool.tile([128, 128], F32, tag="c_ut")
            c_u64 = pool.tile([64, 64], F32, tag="c_u64")
            c_on128 = pool.tile([128, 1], F32, tag="c_on128")
            c_on1r = pool.tile([1, 128], F32R, tag="c_on1r")
            c_io8 = pool.tile([128, E], F32, tag="c_io8")
            c_de8 = pool.tile([128, E], F32, tag="c_de8")
            c_io16 = pool.tile([128, 32], F32, tag="c_io16")
            c_flo = pool.tile([128, 128], F32, tag="c_flo")
            c_fhi = pool.tile([128, 128], F32, tag="c_fhi")
            c_fix = pool.tile([128, 128], F32, tag="c_fix")
            c_tok = pool.tile([128, NT], F32, tag="c_tok")
            c_ev = pool.tile([128, 1], F32, tag="c_ev")
            c_wg = pool.tile([128, D // 128, E], F32, tag="c_wg")
            c_bg = pool.tile([128, E], F32, tag="c_bg")
            c_b1 = pool.tile([128, H // 128], F32, tag="c_b1")
            c_b2 = pool.tile([1, D], F32R, tag="c_b2")
            for t, d in [(c_idn, d_idn), (c_idr, d_idr), (c_ut, d_ut),
                         (c_u64, d_u64), (c_on128, d_on128), (c_on1r, d_on1r),
                         (c_io8, d_io8), (c_de8, d_de8), (c_io16, d_io16),
                         (c_flo, d_flo), (c_fhi, d_fhi), (c_fix, d_fix),
                         (c_tok, d_tok), (c_ev, d_ev), (c_wg, d_wg),
                         (c_bg, d_bg), (c_b1, d_b1), (c_b2, d_b2)]:
                nc.sync.dma_start(t[:], d)

            # routing result buffers
            oh_all = pool.tile([128, NT, E], F32, tag="oh_all")
            eid_all = pool.tile([128, NT], F32, tag="eid_all")
            carry_rep = pool.tile([128, NT * E], F32, tag="carry_rep")
            gidx = pool.tile([128, C // 16], I16, tag="gidx")
            sidx = pool.tile([128, C // 16], I16, tag="sidx")

            d_counts = drp.tile([64, E], F32, tag="d_counts")
            d_carr = drp.tile([64, E], F32, tag="d_carr")

            # =============== PHASE 1: routing ===============
            # pass A: gate + argmax + one-hot per token tile
            for ch in range(16):          # 512-token x chunks
                xc = pool.tile([128, 4, D], F32, tag="xchunk")
                nc.sync.dma_start(
                    xc[:], d_x[ch * 512:(ch + 1) * 512, :].rearrange(
                        "(b p) d -> p b d", p=128))
                for b in range(4):
                    i = 4 * ch + b
                    xT = pool.tile([128, D // 128, 128], F32, tag="xT")
                    for half in range(2):
                        pst = psp.tile([128, 512], F32, tag="psA")
                        for kk in range(4):
                            kb = half * 4 + kk
                            nc.tensor.transpose(
                                pst[:, kk * 128:(kk + 1) * 128],
                                xc[:, b, kb * 128:(kb + 1) * 128], c_idn[:])
                        nc.vector.tensor_copy(xT[:, half * 4:half * 4 + 4, :],
                                              pst[:])
                    psl = psp.tile([128, E], F32, tag="psB")
                    for kb in range(8):
                        nc.tensor.matmul(psl[:], xT[:, kb, :], c_wg[:, kb, :],
                                         start=(kb == 0), stop=(kb == 7))
                    ls = pool.tile([128, E], F32, tag="ls")
                    nc.vector.scalar_tensor_tensor(ls[:], psl[:], 0.0, c_bg[:],
                                                   OP.add, OP.add)
                    mx = pool.tile([128, 1], F32, tag="mx")
                    nc.vector.tensor_reduce(mx[:], ls[:], AX.X, OP.max)
                    t2 = pool.tile([128, E], F32, tag="t2")
                    nc.vector.scalar_tensor_tensor(t2[:], ls[:], mx[:],
                                                   c_de8[:], OP.is_ge, OP.mult)
                    m8 = pool.tile([128, 1], F32, tag="m8")
                    nc.vector.tensor_reduce(m8[:], t2[:], AX.X, OP.max)
                    nc.vector.tensor_scalar(eid_all[:, i:i + 1], m8[:], 8.0,
                                            -1.0, OP.subtract, OP.mult)
                    nc.vector.tensor_scalar(oh_all[:, i, :], c_io8[:],
                                            eid_all[:, i:i + 1], None,
                                            OP.is_equal)

            # counts -> carries -> replicated carries
            psc = psp.tile([1, NT * E], F32, tag="psC")
            nc.tensor.matmul(psc[:], c_on128[:], oh_all[:], start=True,
                             stop=True, skip_group_check=True)
            cf = pool.tile([1, NT * E], F32, tag="cf")
            nc.vector.tensor_copy(cf[:], psc[:])
            nc.sync.dma_start(d_counts[:].rearrange("a b -> (a b)").unsqueeze(0), cf[:])
            csb = pool.tile([64, E], F32, tag="csb")
            nc.sync.dma_start(csb[:], d_counts[:])
            psr = psp.tile([64, E], F32, tag="psC")
            nc.tensor.matmul(psr[:], c_u64[:], csb[:], start=True, stop=True,
                             skip_group_check=True)
            crs = pool.tile([64, E], F32, tag="crs")
            nc.vector.tensor_copy(crs[:], psr[:])
            nc.sync.dma_start(d_carr[:], crs[:])
            cfl = pool.tile([1, NT * E], F32, tag="cf")
            nc.sync.dma_start(cfl[:], d_carr[:].rearrange("a b -> (a b)").unsqueeze(0))
            nc.gpsimd.partition_broadcast(carry_rep[:], cfl[:])
            cr3 = carry_rep[:].rearrange("p (t e) -> p t e", e=E)

            # pass B: positions + index tables
            fin = psp.tile([32, 256], F32, tag="psFin")
            for i in range(NT):
                oh_i = oh_all[:, i, :]
                psq = psp.tile([128, E], F32, tag="psB")
                nc.tensor.matmul(psq[:], c_ut[:], oh_i, start=True, stop=True,
                                 skip_group_check=True)
                j8 = pool.tile([128, E], F32, tag="j8")
                ca = pool.tile([128, 1], F32, tag="ca")
                nc.vector.scalar_tensor_tensor(j8[:], cr3[:, i, :], 0.0, oh_i,
                                               OP.add, OP.mult,
                                               accum_out=ca[:])
                j8b = pool.tile([128, E], F32, tag="j8b")
                pl = pool.tile([128, 1], F32, tag="pl")
                nc.vector.scalar_tensor_tensor(j8b[:], psq[:], 0.0, oh_i,
                                               OP.add, OP.mult,
                                               accum_out=pl[:])
                pm0 = pool.tile([128, 1], F32, tag="pm0")
                nc.vector.tensor_scalar(pm0[:], eid_all[:, i:i + 1], c_ev[:],
                                        1e6, OP.not_equal, OP.mult)
                pm1 = pool.tile([128, 1], F32, tag="pm1")
                nc.vector.scalar_tensor_tensor(pm1[:], pl[:], -1.0, ca[:],
                                               OP.add, OP.add)
                posm = pool.tile([128, 1], F32, tag="posm")
                nc.vector.scalar_tensor_tensor(posm[:], pm0[:], 0.0, pm1[:],
                                               OP.add, OP.add)
                af = pool.tile([128, 128], F32, tag="af")
                nc.vector.tensor_scalar(af[:], c_flo[:], posm[:], None,
                                        OP.is_le)
                rhsb = pool.tile([128, 256], F32, tag="rhsb")
                nc.vector.scalar_tensor_tensor(rhsb[:, 128:256], c_fhi[:],
                                               posm[:], af[:], OP.is_gt,
                                               OP.mult)
                jf = pool.tile([128, 128], F32, tag="jf")
                fnum = pool.tile([128, 1], F32, tag="fnum")
                nc.vector.scalar_tensor_tensor(jf[:], rhsb[:, 128:256], 0.0,
                                               c_fix[:], OP.add, OP.mult,
                                               accum_out=fnum[:])
                lo16 = pool.tile([128, 1], F32, tag="lo16")
                nc.vector.scalar_tensor_tensor(lo16[:], fnum[:], -16.0,
                                               posm[:], OP.mult, OP.add)
                indp = pool.tile([128, 32], F32, tag="indp")
                nc.vector.tensor_scalar(indp[:], c_io16[:], lo16[:], None,
                                        OP.is_equal)
                nc.vector.tensor_scalar(rhsb[:, 0:128], rhsb[:, 128:256],
                                        c_tok[:, i:i + 1], None, OP.mult)
                nc.tensor.matmul(fin[:], indp[:], rhsb[:], start=(i == 0),
                                 stop=(i == NT - 1), skip_group_check=True)

            # finalize idx tables (int16, wrapped [16, C/16] layout,
            # replicated into all 8 Q7-core partition groups; fin already
            # holds two copies on partitions 0-31)
            tsc = pool.tile([32, 128], F32, tag="tsc")
            nc.vector.tensor_scalar(tsc[:], fin[:, 128:256], -8192.0, 8192.0,
                                    OP.mult, OP.add)
            nc.vector.tensor_copy(gidx[0:32, :], fin[:, 0:128])
            nc.vector.scalar_tensor_tensor(sidx[0:32, :], tsc[:], 0.0,
                                           fin[:, 0:128], OP.add, OP.add)
            for q in range(1, 4):
                nc.vector.tensor_copy(gidx[32 * q:32 * q + 32, :],
                                      gidx[0:32, :])
                nc.vector.tensor_copy(sidx[32 * q:32 * q + 32, :],
                                      sidx[0:32, :])

            if debug:
                nc.sync.dma_start(d_dbg_eid, eid_all[:])
                nc.sync.dma_start(d_dbg_cnt, cf[:])
                nc.sync.dma_start(d_dbg_car, carry_rep[:])
                nc.sync.dma_start(d_dbg_gidx, gidx[:])
                nc.sync.dma_start(d_dbg_sidx, sidx[:])

            # =============== PHASE 2: dispatch + MLP + combine ===============
            for mc in range(MC):
                # gather this megachunk's tokens (fp32r rows)
                disp = pool.tile([128, MCT // 128, D], F32R, tag="disp")
                nc.gpsimd.dma_gather(
                    disp[:], d_xr, gidx[:, mc * 64:(mc + 1) * 64], MCT, MCT, D)
                # transpose -> dispT [8][128, MCT]
                dispT = pool.tile([128, D // 128, MCT], F32R, tag="dispT")
                for bb in range(MCT // 128):
                    for half in range(2):
                        pst = psp.tile([128, 512], F32R, tag="psA")
                        for kk in range(4):
                            kb = half * 4 + kk
                            nc.tensor.transpose(
                                pst[:, kk * 128:(kk + 1) * 128],
                                disp[:, bb, kb * 128:(kb + 1) * 128],
                                c_idr[:])
                        for kk in range(4):
                            kb = half * 4 + kk
                            nc.vector.tensor_copy(
                                dispT[:, kb, bb * 128:(bb + 1) * 128],
                                pst[:, kk * 128:(kk + 1) * 128])

                if debug and mc == 0:
                    nc.sync.dma_start(d_dbg_disp, disp[:].bitcast(F32))
                y = pool.tile([128, MCT // 128, D], F32, tag="y")
                for hb in range(NHB):
                    w1b = pool.tile([128, D // 128, HB], F32R, tag="xchunk")
                    nc.sync.dma_start(
                        w1b[:], d_w1[:, hb * HB:(hb + 1) * HB].rearrange(
                            "(kb p) h -> p kb h", p=128))
                    w2b = pool.tile([128, HB // 128, D], F32R, tag="w2b")
                    nc.sync.dma_start(
                        w2b[:], d_w2[hb * HB:(hb + 1) * HB, :].rearrange(
                            "(k p) d -> p k d", p=128))
                    hT = pool.tile([128, HB // 128, MCT], F32R, tag="hT")
                    for m in range(HB // 128):
                        for n in range(MCT // 512):
                            ph = psp.tile([128, 512], F32, tag="psA")
                            for kb in range(D // 128):
                                nc.tensor.matmul(
                                    ph[:], w1b[:, kb, m * 128:(m + 1) * 128],
                                    dispT[:, kb, n * 512:(n + 1) * 512],
                                    start=(kb == 0), stop=(kb == D // 128 - 1))
                            nc.scalar.activation(
                                hT[:, m, n * 512:(n + 1) * 512], ph[:],
                                AF.Relu,
                                bias=c_b1[:, hb * (HB // 128) + m:
                                          hb * (HB // 128) + m + 1],
                                scale=1.0)
                    for b in range(MCT // 128):
                        for n2 in range(D // 512):
                            py = psp.tile([128, 512], F32, tag="psD")
                            for k2 in range(HB // 128):
                                last = k2 == HB // 128 - 1
                                nc.tensor.matmul(
                                    py[:], hT[:, k2, b * 128:(b + 1) * 128],
                                    w2b[:, k2, n2 * 512:(n2 + 1) * 512],
                                    start=(k2 == 0),
                                    stop=(last and hb != 0),
                                    skip_group_check=True)
                            if hb == 0:
                                nc.tensor.matmul(
                                    py[:], c_on1r[:],
                                    c_b2[:, n2 * 512:(n2 + 1) * 512],
                                    start=False, stop=True,
                                    skip_group_check=True)
                            ysl = y[:, b, n2 * 512:(n2 + 1) * 512]
                            if hb == 0:
                                nc.vector.tensor_copy(ysl, py[:])
                            else:
                                nc.vector.scalar_tensor_tensor(
                                    ysl, py[:], 0.0, ysl, OP.add, OP.add)
                # combine
                nc.gpsimd.dma_scatter_add(
                    d_out, y[:], sidx[:, mc * 64:(mc + 1) * 64], MCT, MCT, D)

    nc.compile()
    return nc


def _consts():
    io8 = np.tile(np.arange(E, dtype=np.float32), (128, 1))
    de8 = 8.0 - io8
    io16 = np.tile(np.arange(32, dtype=np.float32) % 16, (128, 1))
    nf = np.arange(128, dtype=np.float32)
    flo = np.tile(16.0 * nf, (128, 1))
    fhi = flo + 16.0
    fix = np.tile(nf, (128, 1))
    tok = (np.arange(NT, dtype=np.float32)[None, :] * 128
           + np.arange(128, dtype=np.float32)[:, None])
    ut = (np.arange(128)[:, None] <= np.arange(128)[None, :]).astype(np.float32)
    u64 = (np.arange(64)[:, None] < np.arange(64)[None, :]).astype(np.float32)
    return {
        "idn": np.eye(128, dtype=np.float32),
        "idr": np.eye(128, dtype=np.float32),
        "ut128": ut, "u64": u64,
        "on128": np.ones((128, 1), np.float32),
        "on1r": np.ones((1, 128), np.float32),
        "io8": io8, "de8": de8, "io16": io16,
        "flo": flo, "fhi": fhi, "fix": fix, "tokid": tok,
    }


def kernel(**inputs):
    from concourse.bass_utils import run_bass_kernel_spmd

    x = np.ascontiguousarray(np.asarray(inputs["x"], dtype=np.float32))
    Wg = np.asarray(inputs["Wg"], dtype=np.float32)
    bg = np.asarray(inputs["bg"], dtype=np.float32)
    W1 = np.asarray(inputs["W1"], dtype=np.float32)
    b1 = np.asarray(inputs["b1"], dtype=np.float32)
    W2 = np.asarray(inputs["W2"], dtype=np.float32)
    b2 = np.asarray(inputs["b2"], dtype=np.float32)

    if "nc" not in _CACHE:
        _CACHE["nc"] = _build()
    nc = _CACHE["nc"]

    xf = x.reshape(N, D)
    consts = _consts()
    wg_l = np.ascontiguousarray(
        Wg.reshape(D // 128, 128, E).transpose(1, 0, 2))
    bg_rep = np.tile(bg[None, :], (128, 1)).astype(np.float32)

    in_maps = []
    for e in range(NCORE):
        m = dict(consts)
        m["x"] = xf
        m["xr"] = xf
        m["wg"] = wg_l
        m["bgrep"] = bg_rep
        m["w1"] = np.ascontiguousarray(W1[e])
        m["w2"] = np.ascontiguousarray(W2[e])
        m["b1l"] = np.ascontiguousarray(
            b1[e].reshape(H // 128, 128).T)
        m["b2r"] = np.ascontiguousarray(b2[e][None, :])
        m["evec"] = np.full((128, 1), float(e), np.float32)
        in_maps.append(m)

    res = run_bass_kernel_spmd(nc, in_maps, core_ids=list(range(NCORE)),
                               trace=False)
    out = np.zeros((N, D), np.float32)
    for e in range(NCORE):
        out += res.results[e]["out"][:N]
    return out.reshape(4, 2048, D)
